# revision 44
# baseline (speedup 1.0000x reference)
"""Fused attention-encoding kernel for Trainium2, 8-core batch-parallel SPMD.

Problem (per batch b of 16, p=1024 tokens, d=512 features):
    A[i,j] = wa.P_i + wb.P_j + (wc*P_i).P_j        (si = wa.P_i cancels in softmax)
    SA     = softmax_j(A)
    attn   = SA @ P
    Pc     = [P, attn]
    out    = sigmoid(Pc@w2) * P + sigmoid(Pc@w3) * tanh(Pc@w1)

Strategy: batch-parallel over 8 cores (2 batches/core). Scores are computed
transposed (S^T[j,i], j on partitions) so sj folds into the exp as a
per-partition activation bias and the attention matmul consumes E=exp(S^T)
directly. The score/attention/rowsum matmuls run in fp8-e4m3 DoubleRow mode
(2 k-tiles per instruction); softmax protects them from quantization noise.
The gate matmuls are precision-graduated per gate (see GATE_MODE): the tanh
z-gate keeps half its P-contraction in bf16, the sigmoid gates run fully in
fp8 DoubleRow. All transposes, scale folds, and sj = P@wb are precomputed
host-side; inputs are shipped in SBUF-layout ([128 partitions, free]).

Perf notes (vs the 106us baseline):
  - input DMAs are spread across the sync/scalar/gpsimd trigger rings so the
    first score operands land ~3us earlier and steady-state loads overlap;
    pt is packed jb-major so each score block's stationary is contiguous.
  - PSUM is tiled as [128,1024] 2-bank tiles: one exp ACT per jb (the ACT
    fixed overhead is per-instruction), wide normalize muls, and the gate
    logits live in 3 wide tiles per ib-pair.
  - the softmax denominator chain (rowsum -> copy -> broadcast-matmul ->
    reciprocal -> normalize) is split per i-half with the jb7 exp, the rs
    copies run on vector+scalar in parallel, and the tensor stream is
    ordered so attn dc2/dc3 + the ib0 gate P-part cover the chain latency.
  - P-residual and the output travel bf16 (the fp8 attn/gate noise dwarfs
    bf16 rounding): -4MiB of HBM traffic per core.

Scale scheme (fp8-e4m3 wants operands ~O(1)):
    pwt8 = (P * wc * 32)^T   -> score PSUM is 32x, exp uses scale=1/32, bias=sjT
    ones = 1/8               -> rowsum PSUM = rs/8, so rb32 = 8/rs
    at8  = attn_unnorm * rb32 = 8*attn (fp8)
    pt16 = (P*32)^T bf16, w16 plain; w8 = w[512:]*4  -> gate PSUM is 32x logits,
    activations use scale=1/32 (bias b*32 added to PSUM before the rescale).
"""

import os
import sys

if "/opt/trn_rl_repo" not in sys.path:
    sys.path.insert(0, "/opt/trn_rl_repo")

from contextlib import ExitStack

import ml_dtypes
import numpy as np

import concourse.bass as bass
import concourse.mybir as mybir
import concourse.tile as tile
from concourse import bacc
from concourse.bass_utils import run_bass_kernel_spmd

B, PL, D = 16, 1024, 512
NCORES = 8
BPC = B // NCORES          # batches per core
NI = PL // 128             # token blocks (i or j): 8
ND = D // 128              # feature chunks: 4
FP32 = mybir.dt.float32
FP32R = mybir.dt.float32r
BF16 = mybir.dt.bfloat16
FP8 = mybir.dt.float8e4
AF = mybir.ActivationFunctionType
DR = mybir.MatmulPerfMode.DoubleRow

NPF8 = ml_dtypes.float8_e4m3
NPBF = ml_dtypes.bfloat16

# Per-gate P-half precision: how many of the 4 contraction chunks run in bf16
# (the rest run as fp8 DoubleRow pairs). The tanh z-gate amplifies logit error
# ~4x more than the sigmoids, so it keeps the bf16 chunks:
#   safe  (2,2,2): rel_err 1.12e-2   zsafe (2,0,0): 1.25e-2   full (0,0,0): 1.58e-2
GATE_MODE = os.environ.get("K_GATE_MODE", "full")
GATE_BF16 = {"safe": (2, 2, 2), "zsafe": (2, 0, 0), "full": (0, 0, 0)}[GATE_MODE]
# per-gate chunk counts in w8 ([4-nbf P-chunks] + [4 attn chunks]) and offsets
W8_NCH = [8 - nbf for nbf in GATE_BF16]
W8_OFF = [sum(W8_NCH[:g]) for g in range(3)]
W16_OFF = [sum(GATE_BF16[:g]) for g in range(3)]
NW16 = sum(GATE_BF16)
NW8 = sum(W8_NCH)

_cache = {}


def _build(with_bias: bool, taps: tuple = ()):
    nc = bacc.Bacc(
        "TRN2", target_bir_lowering=False, debug=False, num_devices=1
    )
    # pq8 packs [pwt01 | pwt23 | ptA(c01 jb-major) | ptB(c23 jb-major)]; the
    # pieces ship as parallel DMAs on the two HWDGE trigger rings (sync +
    # scalar -- the gpsimd SWDGE path is ~2x slower for loads) so the jb0
    # score operands land ~2.6us after the triggers fire.
    # +32 tail cols: sjt (NI fp32 = 32B/partition) rides inside pq8 so it
    # needs no separate DMA (a standalone 4KB transfer costs ~2-4us of fixed
    # completion latency at the head of a queue)
    pq8_d = nc.dram_tensor(
        "pq8", [BPC, 128, 2 * ND * PL + 32], FP8, kind="ExternalInput"
    ).ap()
    if NW16:
        pt16_d = nc.dram_tensor("pt16", [BPC, 128, 2 * PL], BF16, kind="ExternalInput").ap()
    pn8_d = nc.dram_tensor("pn8", [BPC, 128, NI * D], FP8, kind="ExternalInput").ap()
    pn16_d = nc.dram_tensor("pn16", [BPC, 128, NI * D], BF16, kind="ExternalInput").ap()
    if NW16:
        w16_d = nc.dram_tensor("w16", [128, NW16 * D], BF16, kind="ExternalInput").ap()
    w8_d = nc.dram_tensor("w8", [128, NW8 * D], FP8, kind="ExternalInput").ap()
    if with_bias:
        b_d = nc.dram_tensor("b32", [3, D], FP32, kind="ExternalInput").ap()
    out_d = nc.dram_tensor("out", [BPC, PL, D], BF16, kind="ExternalOutput").ap()
    tap_d = {}

    def tap(name, ap, lb=0):
        if lb != 0 or name not in taps:
            return
        t = nc.dram_tensor(
            f"tap_{name}", list(ap.shape), ap.dtype, kind="ExternalOutput"
        ).ap()
        tap_d[name] = t
        nc.sync.dma_start(t, ap)

    with tile.TileContext(nc) as tc, ExitStack() as ctx:
        pool = lambda name, bufs: ctx.enter_context(
            tc.tile_pool(name=name, bufs=bufs)
        )
        const = pool("const", 1)
        wpool = pool("wts", 1)
        pt8p = pool("pt8", 2)
        pt16p = pool("pt16", 2)
        pn8p = pool("pn8", 2)
        pn16p = pool("pn16", 2)
        e8p = pool("e8", 2)
        at8p = pool("at8", 2)
        rb32p = pool("rb32", 2)
        smallp = pool("small", 2)
        op = pool("outs", 3)
        tmpp = pool("tmp", 2)
        gp = pool("gates", 2)
        # 3 wide (2-bank) matmul tiles + 1 wide vec tile = all 8 PSUM banks
        psmm = ctx.enter_context(tc.tile_pool(name="psmm", bufs=3, space="PSUM"))
        psvec = ctx.enter_context(tc.tile_pool(name="psvec", bufs=1, space="PSUM"))

        # --- constants / weights ---
        if NW16:
            w16_sb = wpool.tile([128, NW16 * D], BF16, tag="w16")
        w8_sb = wpool.tile([128, NW8 * D], FP8, tag="w8")

        # DoubleRow ldweights needs the k-tile pair step %16==0, so space the
        # two ones columns 16 elements apart.
        ones8 = const.tile([128, 32], FP8, tag="ones8")
        nc.vector.memset(ones8[:], 0.125)
        ones16 = const.tile([1, 128], BF16, tag="ones16")
        nc.vector.memset(ones16[:], 1.0)
        if with_bias:
            bb = [const.tile([128, D], FP32, tag=f"bias{g}", name=f"bias{g}") for g in range(3)]
            btmp = const.tile([1, 3 * D], FP32, tag="btmp")
            nc.sync.dma_start(btmp[:], b_d.rearrange("g e -> (g e)")[None, :])
            for g in range(3):
                nc.gpsimd.partition_broadcast(
                    bb[g][:], btmp[0:1, g * D : (g + 1) * D]
                )

        # --------- per-lb load issue (spread across the 3 DMA trigger rings) ---------
        HP = ND * PL  # 4096: pwt region size == pt region size

        def issue_loads(lb):
            # The per-queue DMA rate scales with per-partition descriptor
            # size (4KB/p ~ 95-131GB/s, 1KB/p ~ 40GB/s), so the score
            # operands ship as THREE parallel DMAs: pwt01 on sync + ptA on
            # gpsimd cover the dp0 contraction of every jb ~1.3us sooner
            # than a single combined transfer; [pwt23|ptB|sjt] on scalar
            # covers dp1 and the exp bias.
            pq8 = pt8p.tile([128, 2 * HP + 32], FP8, tag="pq8", name=f"pq8_{lb}")
            sjt = pq8[:, 2 * HP : 2 * HP + 32].bitcast(FP32)
            nc.sync.dma_start(pq8[:, 0:2048], pq8_d[lb][:, 0:2048])
            nc.gpsimd.dma_start(pq8[:, 2048:4096], pq8_d[lb][:, 2048:4096])
            nc.scalar.dma_start(
                pq8[:, 4096 : 2 * HP + 32], pq8_d[lb][:, 4096 : 2 * HP + 32]
            )
            pn8 = pn8p.tile([128, NI * D], FP8, tag="pn8", name=f"pn8_{lb}")
            nc.sync.dma_start(pn8[:], pn8_d[lb])
            if NW16:
                pt16 = pt16p.tile([128, 2 * PL], BF16, tag="pt16", name=f"pt16_{lb}")
                nc.scalar.dma_start(pt16[:], pt16_d[lb])
            else:
                pt16 = None
            if lb == 0:
                h8 = (NW8 // 2) * D
                if NW16:
                    nc.scalar.dma_start(w16_sb[:], w16_d)
                nc.scalar.dma_start(w8_sb[:, 0:h8], w8_d[:, 0:h8])
                nc.sync.dma_start(w8_sb[:, h8:], w8_d[:, h8:])
            pn16 = pn16p.tile([128, NI * D], BF16, tag="pn16", name=f"pn16_{lb}")
            return pq8, sjt, pn8, pt16, pn16

        loads = issue_loads(0)

        for lb in range(BPC):
            pq8, sjt, pn8, pt16, pn16 = loads

            # views into the packed pq8 [pwt01 | ptA | pwt23 | ptB]: pwt
            # chunk-major, pt jb-major per chunk-pair
            pwt0v = pq8[:, 0:2048].rearrange("p (c l) -> p c l", l=PL)
            ptAv = pq8[:, 2048:4096].rearrange("p (j c l) -> p j c l", c=2, l=128)
            pwt1v = pq8[:, 4096:6144].rearrange("p (c l) -> p c l", l=PL)
            ptBv = pq8[:, 6144:8192].rearrange("p (j c l) -> p j c l", c=2, l=128)
            pwtv = (pwt0v, pwt1v)
            ptv = (ptAv, ptBv)
            pn8v = pn8.rearrange("p (j d) -> p j d", d=D)

            # ---------- phase B: scores (fp8 DR) + exp + rowsum (fp8 DR) ----------
            e8 = e8p.tile([128, NI * PL], FP8, tag="e8")
            e8v = e8.rearrange("p (j l) -> p j l", l=PL)
            ps_rs = psvec.tile([128, 1024], FP32, tag="psvec", name=f"psrs{lb}")

            def rowsum(jb, ih, start, stop):
                nc.tensor.matmul(
                    ps_rs[0:1, ih * 512 : (ih + 1) * 512],
                    ones8[:, 0:17:16][:, :, None],
                    e8v[:, jb - 1 : jb + 1, ih * 512 : (ih + 1) * 512],
                    start=start,
                    stop=stop,
                    perf_mode=DR,
                )

            for jb in range(NI):
                ps_s = psmm.tile([128, 1024], FP32, tag="psmm", name=f"pss{lb}_{jb}")
                for ih in range(2):
                    for dp in range(2):
                        nc.tensor.matmul(
                            ps_s[:, ih * 512 : (ih + 1) * 512],
                            ptv[dp][:, jb],
                            pwtv[dp][:, :, ih * 512 : (ih + 1) * 512],
                            start=(dp == 0),
                            stop=(dp == 1),
                            perf_mode=DR,
                        )
                if jb < NI - 1:
                    # one wide ACT per jb: the fixed ACT overhead is per-instr
                    nc.scalar.activation(
                        e8v[:, jb, :],
                        ps_s[:],
                        AF.Exp,
                        bias=sjt[:, jb : jb + 1],
                        scale=1.0 / 32.0,
                    )
                else:
                    # jb7 split per i-half so the rowsum/recip chain (which
                    # gates attn dc3 + the ib0 gate tail) starts a half early
                    for ih in range(2):
                        nc.scalar.activation(
                            e8v[:, jb, ih * 512 : (ih + 1) * 512],
                            ps_s[:, ih * 512 : (ih + 1) * 512],
                            AF.Exp,
                            bias=sjt[:, jb : jb + 1],
                            scale=1.0 / 32.0,
                        )
                if jb % 2 == 1 and jb < NI - 1:
                    for ih in range(2):
                        rowsum(jb, ih, start=(jb == 1), stop=False)

            # pn16 (1MB) is only needed by the gate outputs; a tiny gpsimd op
            # that depends on exp jb0 holds its trigger back so the transfer
            # can't steal HBM bandwidth from the critical score loads
            stub = smallp.tile([1, 16], FP8, tag="stub")
            nc.gpsimd.tensor_copy(stub[0:1, :], e8v[0:1, 0, 0:16])
            nc.gpsimd.dma_start(pn16[:], pn16_d[lb])

            # issue next lb's loads now: their trigger-ring slots sit behind
            # this lb's loads but ahead of the phase-D output stores
            if lb + 1 < BPC:
                loads = issue_loads(lb + 1)

            # ---------- phase C: attn^T (fp8 DR) + normalize (-> 8*attn fp8) ----------
            at8 = at8p.tile([128, ND * PL], FP8, tag="at8")
            at8v = at8.rearrange("p (c l) -> p c l", l=PL)
            # two independent tiles so the ih1 copy can't order-block ih0's
            rs16 = [
                smallp.tile([1, 512], BF16, tag=f"rs16{ih}", name=f"rs16{ih}_{lb}")
                for ih in range(2)
            ]
            rb32 = rb32p.tile([128, PL], FP32, tag="rb32")

            def attn_mm(dc, ih, jp, ps):
                nc.tensor.matmul(
                    ps[:, ih * 512 : (ih + 1) * 512],
                    pn8v[:, 2 * jp : 2 * jp + 2, dc * 128 : (dc + 1) * 128],
                    e8v[:, 2 * jp : 2 * jp + 2, ih * 512 : (ih + 1) * 512],
                    start=(jp == 0),
                    stop=(jp == 3),
                    perf_mode=DR,
                )

            # dc0/dc1 jp0-2 only need exps jb0-5, so they run while exp jb6/7
            # drain. The denominator chain (rowsum -> copy -> bcast -> recip
            # -> normalize) is vector-serial, so the bcasts are emitted as
            # early as the copies allow and dc2/dc3 fill the tensor stream
            # underneath it.
            ps_a = {}
            for dc in range(2):
                ps_a[dc] = psmm.tile(
                    [128, 1024], FP32, tag="psmm", name=f"psa{lb}_{dc}"
                )
                for ih in range(2):
                    for jp in range(3):
                        attn_mm(dc, ih, jp, ps_a[dc])
            ps_bc = psvec.tile(
                [128, 1024], FP32, tag="psvec", name=f"psbc{lb}", bufs=1
            )

            def bcast(ih):
                nc.tensor.matmul(
                    ps_bc[:, ih * 512 : (ih + 1) * 512],
                    ones16[:],
                    rs16[ih][0:1, :],
                    start=True,
                    stop=True,
                )

            rowsum(NI - 1, 0, start=False, stop=True)
            nc.vector.tensor_copy(rs16[0][0:1, :], ps_rs[0:1, 0:512])
            attn_mm(0, 0, 3, ps_a[0])
            attn_mm(1, 0, 3, ps_a[1])
            bcast(0)
            rowsum(NI - 1, 1, start=False, stop=True)
            # ih1's copy rides the scalar queue (safe now that the all-tanh
            # gates never force an activation-table reload)
            nc.scalar.copy(rs16[1][0:1, :], ps_rs[0:1, 512:1024])
            attn_mm(0, 1, 3, ps_a[0])
            attn_mm(1, 1, 3, ps_a[1])
            bcast(1)
            nc.vector.reciprocal_approx_fast(out=rb32[:, 0:512], in_=ps_bc[:, 0:512])
            nc.vector.reciprocal_approx_fast(
                out=rb32[:, 512:1024], in_=ps_bc[:, 512:1024]
            )
            # attn dc2 reuses the jb7 score slot, dc3 the psvec slot freed by
            # the recips, so both run under the vector normalize muls
            ps_ad = {2: psmm.tile([128, 1024], FP32, tag="psmm", name=f"psa{lb}_2")}
            for ih in range(2):
                for jp in range(4):
                    attn_mm(2, ih, jp, ps_ad[2])
            nc.vector.tensor_mul(at8v[:, 0, :], ps_a[0][:], rb32[:])
            nc.vector.tensor_mul(at8v[:, 1, :], ps_a[1][:], rb32[:])
            ps_ad[3] = psvec.tile([128, 1024], FP32, tag="psvec", name=f"psd3{lb}")
            for ih in range(2):
                for jp in range(4):
                    attn_mm(3, ih, jp, ps_ad[3])
            nc.vector.tensor_mul(at8v[:, 2, :], ps_ad[2][:], rb32[:])
            nc.vector.tensor_mul(at8v[:, 3, :], ps_ad[3][:], rb32[:])

            tap("sjt", sjt[:], lb)
            tap("e8", e8[:], lb)
            tap("rs16", rs16[:], lb)
            tap("at8", at8[:], lb)
            tap("w8_0", w8_sb[:, 0:2048], lb)

            # ---------- phase D: gates ----------
            # contraction per gate: P chunks (bf16 and/or fp8 DR pairs) + attn
            # chunks as two fp8 DR pairs -- every path lands 32x logits in PSUM.
            # The 6 gate logits of an ib-pair live in 3 wide PSUM tiles.
            if NW16:
                pt16v = pt16.rearrange("p (c l) -> p c l", l=PL)
                w16v = w16_sb.rearrange("p (c d) -> p c d", d=D)
            w8v = w8_sb.rearrange("p (c d) -> p c d", d=D)

            def gate_p_mms(ib, g, ps):
                nbf = GATE_BF16[g]
                for dc in range(nbf):
                    nc.tensor.matmul(
                        ps,
                        pt16v[:, dc, ib * 128 : (ib + 1) * 128],
                        w16v[:, W16_OFF[g] + dc, :],
                        start=(dc == 0),
                        stop=False,
                    )
                for dp in range(nbf // 2, 2):
                    nc.tensor.matmul(
                        ps,
                        ptv[dp][:, ib],
                        w8v[:, W8_OFF[g] + 2 * dp - nbf : W8_OFF[g] + 2 * dp - nbf + 2, :],
                        start=(nbf == 0 and dp == 0),
                        stop=False,
                        perf_mode=DR,
                    )

            def gate_at_mms(ib, g, ps, cp):
                a0 = W8_OFF[g] + 4 - GATE_BF16[g]
                nc.tensor.matmul(
                    ps,
                    at8v[:, 2 * cp : 2 * cp + 2, ib * 128 : (ib + 1) * 128],
                    w8v[:, a0 + 2 * cp : a0 + 2 + 2 * cp, :],
                    start=False,
                    stop=(cp == 1),
                    perf_mode=DR,
                )

            def gate_mms(ib, g, ps):
                gate_p_mms(ib, g, ps)
                gate_at_mms(ib, g, ps, 0)
                gate_at_mms(ib, g, ps, 1)

            def gate_acts_wide(iba, tA, tB, tC, o16pair):
                # one [128,1024] activation per gate covers the whole ib-pair
                # (ACT cost is free-size + fixed overhead, so wide halves the
                # fixed part); DVE stt is 2.5x a plain tensor op, so the
                # output chain sticks to mul/mul/add
                zw = gp.tile([128, 2 * D], FP32, tag="zw")
                rw = gp.tile([128, 2 * D], FP32, tag="rw")
                fw = gp.tile([128, 2 * D], FP32, tag="fw")
                ow = op.tile([128, 2 * D], FP32, tag="ow")
                tw = tmpp.tile([128, 2 * D], FP32, tag="tw")
                sc = 1.0 / 32.0
                # z first: the next pair's matmuls reuse tA's PSUM slot
                nc.scalar.activation(zw[:], tA[:], AF.Tanh, scale=sc)
                nc.scalar.activation(rw[:], tB[:], AF.Sigmoid, scale=sc)
                nc.scalar.activation(fw[:], tC[:], AF.Sigmoid, scale=sc)
                nc.vector.tensor_mul(
                    ow[:], rw[:], pn16[:, iba * D : (iba + 2) * D]
                )
                nc.gpsimd.tensor_mul(tw[:], fw[:], zw[:])
                nc.vector.tensor_add(o16pair[:], ow[:], tw[:])
                # one store per ib-pair: 2KB/partition descriptors run ~2x
                # the rate of 1KB ones; trigger rides the idle sync ring
                nc.sync.dma_start(
                    out_d[lb, iba * 128 : (iba + 2) * 128, :].rearrange(
                        "(b p) c -> p b c", p=128
                    ),
                    o16pair.rearrange("p (b c) -> p b c", c=D),
                )

            def gate_out(ib, slots, last, o16c):
                z_ps, r_ps, f_ps = slots
                if with_bias:
                    # bb holds b*32 so one 1/32 activation rescale covers both
                    for g, ps in enumerate(slots):
                        nc.vector.tensor_add(ps, ps, bb[g][:])
                z32 = gp.tile([128, D], FP32, tag="z32")
                r32 = gp.tile([128, D], FP32, tag="r32")
                f32 = gp.tile([128, D], FP32, tag="f32")
                o32 = op.tile([128, D], FP32, tag="o32")
                o16 = o16c[:, (ib % 2) * D : (ib % 2 + 1) * D]
                t32 = tmpp.tile([128, D], FP32, tag="t32")
                sc = 1.0 / 32.0
                if not last:
                    nc.scalar.activation(r32[:], r_ps, AF.Sigmoid, scale=sc)
                    nc.scalar.activation(z32[:], z_ps, AF.Tanh, scale=sc)
                    nc.scalar.activation(f32[:], f_ps, AF.Sigmoid, scale=sc)
                    nc.vector.tensor_mul(
                        o32[:], r32[:], pn16[:, ib * D : (ib + 1) * D]
                    )
                    nc.vector.tensor_mul(t32[:], f32[:], z32[:])
                    nc.vector.tensor_add(o16, o32[:], t32[:])
                else:
                    # final tile: r/f acts + o32 overlap the z-gate matmuls;
                    # only z -> t -> add trails the last matmul. DMA triggers
                    # from the scalar ring so the drain starts early
                    nc.scalar.activation(r32[:], r_ps, AF.Sigmoid, scale=sc)
                    nc.scalar.activation(f32[:], f_ps, AF.Sigmoid, scale=sc)
                    nc.scalar.activation(z32[:], z_ps, AF.Tanh, scale=sc)
                    nc.vector.tensor_mul(
                        o32[:], r32[:], pn16[:, ib * D : (ib + 1) * D]
                    )
                    nc.vector.tensor_mul(t32[:], f32[:], z32[:])
                    nc.vector.tensor_add(o16, o32[:], t32[:])
                    # the closing pair stores as one 2KB/partition transfer
                    # (2x the descriptor rate of the per-ib 1KB ones)
                    nc.scalar.dma_start(
                        out_d[lb, (ib - 1) * 128 : (ib + 1) * 128, :].rearrange(
                            "(b p) c -> p b c", p=128
                        ),
                        o16c.rearrange("p (b c) -> p b c", c=D),
                    )

            for ibp in range(NI // 2):
                iba, ibb = 2 * ibp, 2 * ibp + 1
                closing = lb == BPC - 1 and ibp == NI // 2 - 1
                tA = psmm.tile([128, 1024], FP32, tag="psmm", name=f"psgA{lb}_{ibp}")
                tB = psmm.tile([128, 1024], FP32, tag="psmm", name=f"psgB{lb}_{ibp}")
                tC = psmm.tile([128, 1024], FP32, tag="psmm", name=f"psgC{lb}_{ibp}")
                if not closing:
                    # same-gate pair packing: tA=[z|z'], tB=[r|r'], tC=[f|f']
                    # so each activation runs once per pair at [128,1024]
                    slot = {
                        (iba, 0): tA[:, 0:512],
                        (ibb, 0): tA[:, 512:1024],
                        (iba, 1): tB[:, 0:512],
                        (ibb, 1): tB[:, 512:1024],
                        (iba, 2): tC[:, 0:512],
                        (ibb, 2): tC[:, 512:1024],
                    }
                    o16pair = op.tile(
                        [128, 2 * D], BF16, tag="o16pair", name=f"o16p{lb}_{ibp}"
                    )
                    for ib in (iba, ibb):
                        if ibp == 0 and ib == iba:
                            # first tile: the P-part has no at8 dependency, so
                            # it is emitted ahead of the attn-chunk mms to
                            # cover the tail of the normalize chain
                            for g in (0, 1, 2):
                                gate_p_mms(ib, g, slot[(ib, g)])
                            for cp in (0, 1):
                                for g in (0, 1, 2):
                                    gate_at_mms(ib, g, slot[(ib, g)], cp)
                        else:
                            for g in (0, 1, 2):
                                gate_mms(ib, g, slot[(ib, g)])
                    gate_acts_wide(iba, tA, tB, tC, o16pair)
                else:
                    slot = {
                        (iba, 0): tA[:, 0:512],
                        (iba, 1): tA[:, 512:1024],
                        (iba, 2): tB[:, 0:512],
                        (ibb, 0): tB[:, 512:1024],
                        (ibb, 1): tC[:, 0:512],
                        (ibb, 2): tC[:, 512:1024],
                    }
                    o16c = op.tile(
                        [128, 2 * D], BF16, tag="o16pair", name=f"o16c{lb}"
                    )
                    for ib in (iba, ibb):
                        last = ib == NI - 1
                        # final tile: z-gate matmuls last, so its PSUM stops
                        # right at the end while r/f activations already ran
                        for g in ((1, 2, 0) if last else (0, 1, 2)):
                            gate_mms(ib, g, slot[(ib, g)])
                        gate_out(
                            ib,
                            (slot[(ib, 0)], slot[(ib, 1)], slot[(ib, 2)]),
                            last,
                            o16c,
                        )

    nc.compile()
    return nc


def _get_nc(with_bias: bool, taps: tuple = ()):
    key = (with_bias, taps)
    if key not in _cache:
        _cache[key] = _build(with_bias, taps)
    return _cache[key]


def _prep_in_maps(P, w_atten, w1, w2, w3, b1, b2, b3):
    P = np.ascontiguousarray(np.asarray(P, dtype=np.float32))
    w_atten = np.asarray(w_atten, dtype=np.float32)
    wb = w_atten[D : 2 * D]
    wc = w_atten[2 * D :]

    # transposed layouts: arr[b, p, c, l] = P[b, l, c*128+p]
    PT = np.ascontiguousarray(
        P.reshape(B, PL, ND, 128).transpose(0, 3, 2, 1)
    )  # [B, 128, ND, PL]
    pt8c = PT.astype(NPF8)  # [B, 128, ND, PL]
    pwt8c = (PT * (wc.reshape(ND, 128).T[:, :, None] * 32.0)).astype(NPF8)
    # packed [pwt chunk-major | ptA (chunks 0/1 jb-major) | ptB (chunks 2/3)]
    ptjb = pt8c.reshape(B, 128, 2, 2, NI, 128)  # [b, p, pair, c, jb, l]
    ptA = np.ascontiguousarray(ptjb[:, :, 0].transpose(0, 1, 3, 2, 4)).reshape(
        B, 128, NI * 256
    )
    ptB = np.ascontiguousarray(ptjb[:, :, 1].transpose(0, 1, 3, 2, 4)).reshape(
        B, 128, NI * 256
    )
    pw = pwt8c.reshape(B, 128, ND * PL)
    sj = P @ wb  # [B, PL]
    sjt = np.ascontiguousarray(sj.reshape(B, NI, 128).transpose(0, 2, 1)).astype(
        np.float32
    )  # [B, 128, NI]
    sjt8 = sjt.view(np.uint8).view(NPF8)  # fp32 bytes riding as fp8 cols
    pq8 = np.concatenate(
        [pw[:, :, 0 : 2 * PL], ptA, pw[:, :, 2 * PL :], ptB, sjt8], axis=2
    )  # [B, 128, 8224] = [pwt01 | ptA | pwt23 | ptB | sjt-bytes]
    if NW16:
        pt16 = (PT[:, :, :2] * 32.0).astype(NPBF).reshape(B, 128, 2 * PL)
    # row-block layout [B, 128, NI*D]: arr[b, p, i*D+k] = P[b, i*128+p, k]
    PN = np.ascontiguousarray(
        P.reshape(B, NI, 128, D).transpose(0, 2, 1, 3)
    ).reshape(B, 128, NI * D)
    pn8 = PN.astype(NPF8)
    pn16 = PN.astype(NPBF)

    W = np.stack([np.asarray(w, dtype=np.float32) for w in (w1, w2, w3)])  # [3, 2D, D]
    # per gate g: w16 holds P-chunks 0..nbf-1 plain (pt16 carries the x32);
    # w8 holds [P-chunks nbf..3 @ x32, attn-chunks 0-3 @ x4], tightly packed
    w16_parts, w8_parts = [], []
    for g in range(3):
        nbf = GATE_BF16[g]
        Wg = W[g].reshape(2 * ND, 128, D)  # contraction chunks
        w16_parts.append(Wg[:nbf].transpose(1, 0, 2))
        w8_parts.append(
            np.concatenate([Wg[nbf:ND] * 32.0, Wg[ND:] * 4.0], axis=0).transpose(1, 0, 2)
        )
    if NW16:
        w16 = np.ascontiguousarray(np.concatenate(w16_parts, axis=1)).astype(
            NPBF
        ).reshape(128, NW16 * D)
    w8 = np.ascontiguousarray(np.concatenate(w8_parts, axis=1)).astype(NPF8).reshape(
        128, NW8 * D
    )

    biases = np.stack([np.asarray(b, dtype=np.float32) for b in (b1, b2, b3)])
    with_bias = bool(np.any(biases))

    base = {"w8": w8}
    if NW16:
        base["w16"] = w16
    if with_bias:
        base["b32"] = biases * 32.0
    in_maps = []
    for c in range(NCORES):
        s = slice(c * BPC, (c + 1) * BPC)
        m = dict(base)
        m["pq8"] = np.ascontiguousarray(pq8[s])
        if NW16:
            m["pt16"] = pt16[s]
        m["pn8"] = pn8[s]
        m["pn16"] = pn16[s]
        in_maps.append(m)
    return in_maps, with_bias


def run(P, w_atten, w1, w2, w3, b1, b2, b3, trace=False, taps=()):
    in_maps, with_bias = _prep_in_maps(P, w_atten, w1, w2, w3, b1, b2, b3)
    nc = _get_nc(with_bias, tuple(taps))
    res = run_bass_kernel_spmd(
        nc, in_maps, core_ids=list(range(NCORES)), trace=trace
    )
    out = np.concatenate(
        [res.results[c]["out"].astype(np.float32) for c in range(NCORES)], axis=0
    )
    return out, res


def kernel(P, w_atten, w1, w2, w3, b1, b2, b3):
    out, _ = run(P, w_atten, w1, w2, w3, b1, b2, b3)
    return out
